# revision 5
# baseline (speedup 1.0000x reference)
"""Sliding-window GQA attention (softcap) on 8 trn2 NeuronCores.

Problem shapes (hardcoded):
  Q [1, 32, 2048, 128] bf16, K/V [1, 8, 2048, 128] bf16 -> out [1, 32, 2048, 128] f32
  causal, window_left=256, softcap=30, scale=1/sqrt(128), GQA group=4.

Sharding: core c owns kv-head c and query heads [4c, 4c+4). Each (b, h_kv)
slice is fully independent -> no collectives.

Per-core kernel, v2 (ACT-pipelined):
  ACT (tanh+exp over all 5760 score cols/head, ~11.6us/head busy) is the
  bottleneck engine; the schedule keeps it streaming:
  - a warmup activation at t=0 pulls the ~2.7us ACT table load into the DMA
    lead-in; K^T/Q^T transposes go on two HWDGE rings in parallel.
  - ACT order per head: [tanh g0..g3, exp(strips 0-7), tanh g4..g7,
    exp(strips 8-15)]; the two big exp chunks amortize the ~330-cycle
    per-instruction overhead.
  - software-pipelined carry: head h's second-half mask/PV/normalize/DMA is
    emitted inside head h+1's loop, so on the TensorE queue it lands in the
    exp-A(h+1) window and never delays the QK matmuls that feed tanh.
  - scores stay in the transposed S^T[k, q] layout (strip per key-block) so
    post-softmax P^T is directly the PV lhsT; softcap bounds scores at +-30
    so exp uses the constant shift 30 (no row max).
  - PV accumulates O (+ row-sum via a ones-column in V) into [128,4,256]
    psum quads; normalize is one recip + one broadcast-mul per quad.
  - the last head's tail runs exp/PV/normalize/DMA per strip-pair.
"""

import math
from contextlib import ExitStack

import numpy as np

import concourse.bacc as bacc
import concourse.bass as bass
import concourse.mybir as mybir
import concourse.tile as tile
from concourse.bass import MemorySpace
from concourse.bass_utils import run_bass_kernel_spmd

BF16 = mybir.dt.bfloat16
F32 = mybir.dt.float32

N_CORES = 8
HQ_PER_CORE = 4  # GQA group size
SQ = 2048
D = 128
NB = SQ // 128  # 16 key/query blocks
SCALE = 1.0 / math.sqrt(128.0)
SOFTCAP = 30.0

# strip widths: key-block kb sees q-columns [kb*128, kb*128 + W[kb])
WIDTHS = [min(384, SQ - kb * 128) for kb in range(NB)]
OFFS = [sum(WIDTHS[:kb]) for kb in range(NB)]
TOT = sum(WIDTHS)  # 5760 score columns per head


def build_attention(nc: bass.Bass, q, k, v, out):
    """q [4,2048,128] bf16; k,v [2048,128] bf16; out [4,2048,128] f32 (DRAM APs)."""
    with ExitStack() as ctx:
        tc = ctx.enter_context(tile.TileContext(nc))
        consts = ctx.enter_context(tc.tile_pool(name="consts", bufs=1))
        qt_pool = ctx.enter_context(tc.tile_pool(name="qt", bufs=3))
        t_pool = ctx.enter_context(tc.tile_pool(name="tbuf", bufs=2))
        p_pool = ctx.enter_context(tc.tile_pool(name="pbuf", bufs=2))
        o_pool = ctx.enter_context(tc.tile_pool(name="obuf", bufs=2))
        r_pool = ctx.enter_context(tc.tile_pool(name="rtile", bufs=4))
        spsum = ctx.enter_context(
            tc.tile_pool(name="spsum", bufs=2, space=MemorySpace.PSUM)
        )
        opsum = ctx.enter_context(
            tc.tile_pool(name="opsum", bufs=2, space=MemorySpace.PSUM)
        )

        # ---- t=0: ACT table-load warmup (exp set includes tanh). The memset
        # runs on gpsimd so the warmup has no dependency on DMA or DVE.
        warm = consts.tile([128, 2], F32)
        nc.gpsimd.memset(warm[:, 0:1], 0.0)
        nc.scalar.activation(
            out=warm[:, 1:2], in_=warm[:, 0:1],
            func=mybir.ActivationFunctionType.Exp,
        )

        # ---- input staging: Q^T head 0 on the sync ring, K^T + later heads
        # on the scalar ring (parallel rings; all transposes back-to-back per
        # ring to avoid xbar mode flips).
        kt = consts.tile([128, SQ], BF16)
        qts = [
            qt_pool.tile([128, SQ], BF16, name=f"qt{h}", tag="qt")
            for h in range(HQ_PER_CORE)
        ]
        nc.sync.dma_start_transpose(out=qts[0], in_=q[0])
        nc.scalar.dma_start_transpose(out=kt, in_=k)
        nc.scalar.dma_start_transpose(out=qts[1], in_=q[1])

        def kt_blk(kb):
            return kt[:, kb * 128 : (kb + 1) * 128]

        def qt_rhs(h, kb, w):
            return qts[h][:, kb * 128 : kb * 128 + w]

        # V blocks + ones column (sync ring, after the transposes)
        vt = consts.tile([128, NB, 129], BF16)
        nc.vector.memset(vt[:, :, 128:129], 1.0)
        nc.sync.dma_start(
            out=vt[:, :, 0:128], in_=v.rearrange("(t p) d -> p t d", p=128)
        )
        # band masks, combined [128, 2, 128]: slot 0 keeps c >= kr (upper tri
        # incl diag, strip block 0), slot 1 keeps c <= kr (lower tri, block 2)
        muL = consts.tile([128, 2, 128], BF16)
        nc.gpsimd.memset(muL, 1.0)
        nc.gpsimd.affine_select(
            out=muL[:, 0, :], in_=muL[:, 0, :], compare_op=mybir.AluOpType.is_ge,
            fill=0.0, base=0, pattern=[[1, 128]], channel_multiplier=-1,
        )
        nc.gpsimd.affine_select(
            out=muL[:, 1, :], in_=muL[:, 1, :], compare_op=mybir.AluOpType.is_ge,
            fill=0.0, base=0, pattern=[[-1, 128]], channel_multiplier=1,
        )
        negcap = consts.tile([128, 1], F32)
        nc.gpsimd.memset(negcap, -SOFTCAP)

        def qk_group(h, g):
            """Scores for strips (2g, 2g+1) -> one 2-bank psum tile."""
            kb0, kb1 = 2 * g, 2 * g + 1
            sp = spsum.tile([128, 1024], F32, name="sp", tag="sp")
            for j, kb in enumerate((kb0, kb1)):
                w = WIDTHS[kb]
                nc.tensor.matmul(
                    out=sp[:, j * 512 : j * 512 + w],
                    lhsT=kt_blk(kb),
                    rhs=qt_rhs(h, kb, w),
                    start=True,
                    stop=True,
                )
            return sp

        def tanh_group(g, sp, tbuf):
            kb0, kb1 = 2 * g, 2 * g + 1
            if WIDTHS[kb0] == WIDTHS[kb1]:
                w = WIDTHS[kb0]
                src = sp[:].rearrange("p (g x) -> p g x", g=2)[:, :, 0:w]
                dst = tbuf[:, OFFS[kb0] : OFFS[kb0] + 2 * w].rearrange(
                    "p (g x) -> p g x", g=2
                )
                nc.scalar.activation(
                    out=dst, in_=src,
                    func=mybir.ActivationFunctionType.Tanh,
                    scale=SCALE / SOFTCAP,
                )
            else:
                for j, kb in enumerate((kb0, kb1)):
                    w = WIDTHS[kb]
                    nc.scalar.activation(
                        out=tbuf[:, OFFS[kb] : OFFS[kb] + w],
                        in_=sp[:, j * 512 : j * 512 + w],
                        func=mybir.ActivationFunctionType.Tanh,
                        scale=SCALE / SOFTCAP,
                    )

        def exp_chunk(tbuf, pbuf, lo, hi):
            nc.scalar.activation(
                out=pbuf[:, lo:hi], in_=tbuf[:, lo:hi],
                func=mybir.ActivationFunctionType.Exp,
                scale=SOFTCAP, bias=negcap,
            )

        def mask_strips(pbuf, kb_lo, kb_hi):
            """Zero invalid triangles of strips [kb_lo, kb_hi)."""
            for kb in range(kb_lo, kb_hi):
                off = OFFS[kb]
                if WIDTHS[kb] == 384:
                    view = pbuf[:, off : off + 384].rearrange(
                        "p (a x) -> p a x", x=128
                    )[:, ::2, :]
                    nc.vector.tensor_mul(out=view, in0=view, in1=muL)
                else:
                    nc.vector.tensor_mul(
                        out=pbuf[:, off : off + 128],
                        in0=pbuf[:, off : off + 128],
                        in1=muL[:, 0, :],
                    )

        def pv_qb(pbuf, otile, qb):
            """Accumulate O[qb] (+ rowsum col 128) into otile slot qb%4."""
            kbs = [kb for kb in (qb - 2, qb - 1, qb) if kb >= 0]
            for kb in kbs:
                j = qb - kb
                nc.tensor.matmul(
                    out=otile[:, qb % 4, 0:129],
                    lhsT=pbuf[:, OFFS[kb] + j * 128 : OFFS[kb] + (j + 1) * 128],
                    rhs=vt[:, kb, :],
                    start=(kb == kbs[0]),
                    stop=(kb == qb),
                )

        def normalize_quad(otile, obuf, quad):
            rt = r_pool.tile([128, 4], F32)
            nc.vector.reciprocal(out=rt, in_=otile[:, :, 128])
            nc.vector.tensor_mul(
                out=obuf[:, 4 * quad : 4 * quad + 4, :],
                in0=otile[:, :, 0:128],
                in1=rt.to_broadcast([128, 4, 128]),
            )

        def half_a(hs):
            """mask/PV/normalize/DMA for strips 0-7 of head state hs."""
            pbuf, obuf, out_v = hs["pbuf"], hs["obuf"], hs["out_v"]
            mask_strips(pbuf, 0, 8)
            ot0 = opsum.tile([128, 4, 256], F32, name="ot", tag="ot")
            for qb in range(4):
                pv_qb(pbuf, ot0, qb)
            ot1 = opsum.tile([128, 4, 256], F32, name="ot", tag="ot")
            for qb in range(4, 8):
                pv_qb(pbuf, ot1, qb)
            normalize_quad(ot0, obuf, 0)
            normalize_quad(ot1, obuf, 1)
            nc.sync.dma_start(out=out_v[:, 0:8, :], in_=obuf[:, 0:8, :])

        def half_b(hs):
            """mask/PV/normalize/DMA for strips 8-15 of head state hs."""
            pbuf, obuf, out_v = hs["pbuf"], hs["obuf"], hs["out_v"]
            mask_strips(pbuf, 8, NB)
            ot2 = opsum.tile([128, 4, 256], F32, name="ot", tag="ot")
            for qb in range(8, 12):
                pv_qb(pbuf, ot2, qb)
            ot3 = opsum.tile([128, 4, 256], F32, name="ot", tag="ot")
            for qb in range(12, NB):
                pv_qb(pbuf, ot3, qb)
            normalize_quad(ot2, obuf, 2)
            normalize_quad(ot3, obuf, 3)
            nc.sync.dma_start(out=out_v[:, 8:NB, :], in_=obuf[:, 8:NB, :])

        pending = {}
        carry = None  # head state whose second half still needs mask/PV/out
        for h in range(HQ_PER_CORE):
            if h + 2 < HQ_PER_CORE:
                nc.scalar.dma_start_transpose(out=qts[h + 2], in_=q[h + 2])
            hs = {
                "h": h,
                "tbuf": t_pool.tile([128, TOT], F32, name="tbuf", tag="tbuf"),
                "pbuf": p_pool.tile([128, TOT], BF16, name="pbuf", tag="pbuf"),
                "obuf": o_pool.tile([128, NB, 128], F32, name="obuf", tag="obuf"),
                "out_v": out[h].rearrange("(qb p) d -> p qb d", p=128),
            }
            # ---- ACT first half: tanh g0..g3, exp(strips 0-7)
            for g in range(4):
                sp = pending.pop((h, g), None)
                if sp is None:
                    sp = qk_group(h, g)
                tanh_group(g, sp, hs["tbuf"])
            exp_chunk(hs["tbuf"], hs["pbuf"], 0, OFFS[8])
            # TE work for the exp-A window: previous head's second half
            if carry is not None:
                half_b(carry)
                carry = None
            # ---- ACT second half: tanh g4..g7, exp(strips 8-15)
            for g in range(4, 8):
                tanh_group(g, qk_group(h, g), hs["tbuf"])
            # hoist next head's first two QK groups (TE work + early tanh input)
            if h + 1 < HQ_PER_CORE:
                pending[(h + 1, 0)] = qk_group(h + 1, 0)
                pending[(h + 1, 1)] = qk_group(h + 1, 1)
            if h < HQ_PER_CORE - 1:
                exp_chunk(hs["tbuf"], hs["pbuf"], OFFS[8], TOT)
                # TE work for the exp-B window: this head's first half
                half_a(hs)
                carry = hs
            else:
                # last head: first half, then a fine-grained tail per strip-pair
                half_a(hs)
                pbuf, obuf, out_v = hs["pbuf"], hs["obuf"], hs["out_v"]
                ots = {2: None, 3: None}
                for g in range(4, 8):
                    kb0, kb1 = 2 * g, 2 * g + 1
                    exp_chunk(hs["tbuf"], pbuf, OFFS[kb0],
                              OFFS[kb1] + WIDTHS[kb1])
                    mask_strips(pbuf, kb0, kb1 + 1)
                    for qb in (kb0, kb1):
                        quad = qb // 4
                        if ots[quad] is None:
                            ots[quad] = opsum.tile(
                                [128, 4, 256], F32, name="ot", tag="ot"
                            )
                        pv_qb(pbuf, ots[quad], qb)
                    if g == 5:
                        normalize_quad(ots[2], obuf, 2)
                        nc.sync.dma_start(
                            out=out_v[:, 8:12, :], in_=obuf[:, 8:12, :]
                        )
                normalize_quad(ots[3], obuf, 3)
                nc.sync.dma_start(out=out_v[:, 12:14, :], in_=obuf[:, 12:14, :])
                nc.sync.dma_start(out=out_v[:, 14:NB, :], in_=obuf[:, 14:NB, :])
    return nc


_CACHED = None


def _build():
    global _CACHED
    if _CACHED is None:
        nc = bacc.Bacc()
        q = nc.dram_tensor("q", [HQ_PER_CORE, SQ, D], BF16, kind="ExternalInput")
        k = nc.dram_tensor("k", [SQ, D], BF16, kind="ExternalInput")
        v = nc.dram_tensor("v", [SQ, D], BF16, kind="ExternalInput")
        out = nc.dram_tensor("out", [HQ_PER_CORE, SQ, D], F32, kind="ExternalOutput")
        build_attention(nc, q[:], k[:], v[:], out[:])
        nc.finalize()
        _CACHED = nc
    return _CACHED


def make_in_maps(Q, K, V):
    import ml_dtypes

    Qn = np.asarray(Q).astype(ml_dtypes.bfloat16).reshape(32, SQ, D)
    Kn = np.asarray(K).astype(ml_dtypes.bfloat16).reshape(8, SQ, D)
    Vn = np.asarray(V).astype(ml_dtypes.bfloat16).reshape(8, SQ, D)
    return [
        {
            "q": np.ascontiguousarray(Qn[4 * c : 4 * c + 4]),
            "k": np.ascontiguousarray(Kn[c]),
            "v": np.ascontiguousarray(Vn[c]),
        }
        for c in range(N_CORES)
    ]


def kernel(Q, K, V):
    nc = _build()
    in_maps = make_in_maps(Q, K, V)
    res = run_bass_kernel_spmd(nc, in_maps, list(range(N_CORES))).results
    out = np.stack([res[c]["out"] for c in range(N_CORES)])  # [8,4,2048,128]
    return out.reshape(1, 32, SQ, D).astype(np.float32)


# revision 8
# speedup vs baseline: 1.1259x; 1.1259x over previous
"""Sliding-window GQA attention (softcap) on 8 trn2 NeuronCores.

Problem shapes (hardcoded):
  Q [1, 32, 2048, 128] bf16, K/V [1, 8, 2048, 128] bf16 -> out [1, 32, 2048, 128] f32
  causal, window_left=256, softcap=30, scale=1/sqrt(128), GQA group=4.

Sharding: core c owns kv-head c and query heads [4c, 4c+4). Each (b, h_kv)
slice is fully independent -> no collectives.

Per-core kernel, v2 (ACT-pipelined):
  ACT (tanh+exp over all 5760 score cols/head, ~11.6us/head busy) is the
  bottleneck engine; the schedule keeps it streaming:
  - a warmup activation at t=0 pulls the ~2.7us ACT table load into the DMA
    lead-in; K^T/Q^T transposes go on two HWDGE rings in parallel.
  - ACT order per head: [tanh g0..g3, exp(strips 0-7), tanh g4..g7,
    exp(strips 8-15)]; the two big exp chunks amortize the ~330-cycle
    per-instruction overhead.
  - software-pipelined carry: head h's second-half mask/PV/normalize/DMA is
    emitted inside head h+1's loop, so on the TensorE queue it lands in the
    exp-A(h+1) window and never delays the QK matmuls that feed tanh.
  - scores stay in the transposed S^T[k, q] layout (strip per key-block) so
    post-softmax P^T is directly the PV lhsT; softcap bounds scores at +-30
    so exp uses the constant shift 30 (no row max).
  - PV accumulates O (+ row-sum via a ones-column in V) into [128,4,256]
    psum quads; normalize is one recip + one broadcast-mul per quad.
  - the last head's tail runs exp/PV/normalize/DMA per strip-pair.
"""

import math
from contextlib import ExitStack

import numpy as np

import concourse.bacc as bacc
import concourse.bass as bass
import concourse.mybir as mybir
import concourse.tile as tile
from concourse.bass import MemorySpace
from concourse.bass_utils import run_bass_kernel_spmd

BF16 = mybir.dt.bfloat16
F32 = mybir.dt.float32

N_CORES = 8
HQ_PER_CORE = 4  # GQA group size
SQ = 2048
D = 128
NB = SQ // 128  # 16 key/query blocks
SCALE = 1.0 / math.sqrt(128.0)
SOFTCAP = 30.0

# strip widths: key-block kb sees q-columns [kb*128, kb*128 + W[kb])
WIDTHS = [min(384, SQ - kb * 128) for kb in range(NB)]
OFFS = [sum(WIDTHS[:kb]) for kb in range(NB)]
TOT = sum(WIDTHS)  # 5760 score columns per head


def build_attention(nc: bass.Bass, q, k, v, out):
    """q [4,2048,128] bf16; k,v [2048,128] bf16; out [4,2048,128] f32 (DRAM APs)."""
    with ExitStack() as ctx:
        tc = ctx.enter_context(tile.TileContext(nc))
        consts = ctx.enter_context(tc.tile_pool(name="consts", bufs=1))
        qt_pool = ctx.enter_context(tc.tile_pool(name="qt", bufs=3))
        t_pool = ctx.enter_context(tc.tile_pool(name="tbuf", bufs=2))
        p_pool = ctx.enter_context(tc.tile_pool(name="pbuf", bufs=2))
        o_pool = ctx.enter_context(tc.tile_pool(name="obuf", bufs=2))
        r_pool = ctx.enter_context(tc.tile_pool(name="rtile", bufs=4))
        spsum = ctx.enter_context(
            tc.tile_pool(name="spsum", bufs=2, space=MemorySpace.PSUM)
        )
        opsum = ctx.enter_context(
            tc.tile_pool(name="opsum", bufs=2, space=MemorySpace.PSUM)
        )

        # ---- t=0: ACT table-load warmup (exp set includes tanh). The memset
        # runs on DVE (gpsimd's first instruction pays a ~6us IRAM load).
        warm = consts.tile([128, 2], F32)
        nc.vector.memset(warm[:, 0:1], 0.0)
        nc.scalar.activation(
            out=warm[:, 1:2], in_=warm[:, 0:1],
            func=mybir.ActivationFunctionType.Exp,
        )
        negcap = consts.tile([128, 1], F32)
        nc.vector.memset(negcap, -SOFTCAP)

        # ---- input staging: K^T is the single scalar-ring trigger (issued at
        # t=0 while the ACT queue is otherwise empty -- mid-stream triggers on
        # the scalar queue cost ~2us and stall the activation pipeline).
        # Q^T transposes + V + output stores all go on the sync ring.
        kt = consts.tile([128, SQ], BF16)
        qts = [
            qt_pool.tile([128, SQ], BF16, name=f"qt{h}", tag="qt")
            for h in range(HQ_PER_CORE)
        ]
        nc.scalar.dma_start_transpose(out=kt, in_=k)
        nc.sync.dma_start_transpose(out=qts[0], in_=q[0])
        nc.sync.dma_start_transpose(out=qts[1], in_=q[1])

        def kt_blk(kb):
            return kt[:, kb * 128 : (kb + 1) * 128]

        def qt_rhs(h, kb, w):
            return qts[h][:, kb * 128 : kb * 128 + w]

        # V blocks + ones column (sync ring, after the transposes)
        vt = consts.tile([128, NB, 129], BF16)
        nc.vector.memset(vt[:, :, 128:129], 1.0)
        nc.sync.dma_start(
            out=vt[:, :, 0:128], in_=v.rearrange("(t p) d -> p t d", p=128)
        )
        # band masks, combined [128, 2, 128]: slot 0 keeps c >= kr (upper tri
        # incl diag, strip block 0), slot 1 keeps c <= kr (lower tri, block 2)
        muL = consts.tile([128, 2, 128], BF16)
        nc.gpsimd.memset(muL, 1.0)
        nc.gpsimd.affine_select(
            out=muL[:, 0, :], in_=muL[:, 0, :], compare_op=mybir.AluOpType.is_ge,
            fill=0.0, base=0, pattern=[[1, 128]], channel_multiplier=-1,
        )
        nc.gpsimd.affine_select(
            out=muL[:, 1, :], in_=muL[:, 1, :], compare_op=mybir.AluOpType.is_ge,
            fill=0.0, base=0, pattern=[[-1, 128]], channel_multiplier=1,
        )

        def qk_group(h, g):
            """Scores for strips (2g, 2g+1) -> one 2-bank psum tile."""
            kb0, kb1 = 2 * g, 2 * g + 1
            sp = spsum.tile([128, 1024], F32, name="sp", tag="sp")
            for j, kb in enumerate((kb0, kb1)):
                w = WIDTHS[kb]
                nc.tensor.matmul(
                    out=sp[:, j * 512 : j * 512 + w],
                    lhsT=kt_blk(kb),
                    rhs=qt_rhs(h, kb, w),
                    start=True,
                    stop=True,
                )
            return sp

        def tanh_group(g, sp, tbuf):
            kb0, kb1 = 2 * g, 2 * g + 1
            if WIDTHS[kb0] == WIDTHS[kb1]:
                w = WIDTHS[kb0]
                src = sp[:].rearrange("p (g x) -> p g x", g=2)[:, :, 0:w]
                dst = tbuf[:, OFFS[kb0] : OFFS[kb0] + 2 * w].rearrange(
                    "p (g x) -> p g x", g=2
                )
                nc.scalar.activation(
                    out=dst, in_=src,
                    func=mybir.ActivationFunctionType.Tanh,
                    scale=SCALE / SOFTCAP,
                )
            else:
                for j, kb in enumerate((kb0, kb1)):
                    w = WIDTHS[kb]
                    nc.scalar.activation(
                        out=tbuf[:, OFFS[kb] : OFFS[kb] + w],
                        in_=sp[:, j * 512 : j * 512 + w],
                        func=mybir.ActivationFunctionType.Tanh,
                        scale=SCALE / SOFTCAP,
                    )

        def exp_chunk(tbuf, pbuf, lo, hi):
            nc.scalar.activation(
                out=pbuf[:, lo:hi], in_=tbuf[:, lo:hi],
                func=mybir.ActivationFunctionType.Exp,
                scale=SOFTCAP, bias=negcap,
            )

        def mask_strips(pbuf, kb_lo, kb_hi):
            """Zero invalid triangles of strips [kb_lo, kb_hi)."""
            for kb in range(kb_lo, kb_hi):
                off = OFFS[kb]
                if WIDTHS[kb] == 384:
                    view = pbuf[:, off : off + 384].rearrange(
                        "p (a x) -> p a x", x=128
                    )[:, ::2, :]
                    nc.vector.tensor_mul(out=view, in0=view, in1=muL)
                else:
                    nc.vector.tensor_mul(
                        out=pbuf[:, off : off + 128],
                        in0=pbuf[:, off : off + 128],
                        in1=muL[:, 0, :],
                    )

        def pv_qb(pbuf, otile, qb):
            """Accumulate O[qb] (+ rowsum col 128) into otile slot qb%4."""
            kbs = [kb for kb in (qb - 2, qb - 1, qb) if kb >= 0]
            for kb in kbs:
                j = qb - kb
                nc.tensor.matmul(
                    out=otile[:, qb % 4, 0:129],
                    lhsT=pbuf[:, OFFS[kb] + j * 128 : OFFS[kb] + (j + 1) * 128],
                    rhs=vt[:, kb, :],
                    start=(kb == kbs[0]),
                    stop=(kb == qb),
                )

        def normalize_quad(otile, obuf, quad):
            rt = r_pool.tile([128, 4], F32)
            nc.vector.reciprocal(out=rt, in_=otile[:, :, 128])
            nc.vector.tensor_mul(
                out=obuf[:, 4 * quad : 4 * quad + 4, :],
                in0=otile[:, :, 0:128],
                in1=rt.to_broadcast([128, 4, 128]),
            )

        def pv_half(hs, half):
            """PV/normalize/DMA for strips 8*half..8*half+8 (mask already done)."""
            pbuf, obuf, out_v = hs["pbuf"], hs["obuf"], hs["out_v"]
            qb0 = 8 * half
            ota = opsum.tile([128, 4, 256], F32, name="ot", tag="ot")
            for qb in range(qb0, qb0 + 4):
                pv_qb(pbuf, ota, qb)
            otb = opsum.tile([128, 4, 256], F32, name="ot", tag="ot")
            for qb in range(qb0 + 4, qb0 + 8):
                pv_qb(pbuf, otb, qb)
            normalize_quad(ota, obuf, 2 * half)
            normalize_quad(otb, obuf, 2 * half + 1)
            nc.sync.dma_start(
                out=out_v[:, qb0 : qb0 + 8, :], in_=obuf[:, qb0 : qb0 + 8, :]
            )

        pending = {}
        carry = None  # head state whose second half still needs PV/out
        for h in range(HQ_PER_CORE):
            if h + 2 < HQ_PER_CORE:
                nc.sync.dma_start_transpose(out=qts[h + 2], in_=q[h + 2])
            hs = {
                "h": h,
                "tbuf": t_pool.tile([128, TOT], F32, name="tbuf", tag="tbuf"),
                "pbuf": p_pool.tile([128, TOT], BF16, name="pbuf", tag="pbuf"),
                "obuf": o_pool.tile([128, NB, 128], F32, name="obuf", tag="obuf"),
                "out_v": out[h].rearrange("(qb p) d -> p qb d", p=128),
            }
            # ---- ACT first half: tanh g0..g3, exp(strips 0-7), mask right away
            for g in range(4):
                sp = pending.pop((h, g), None)
                if sp is None:
                    sp = qk_group(h, g)
                tanh_group(g, sp, hs["tbuf"])
            exp_chunk(hs["tbuf"], hs["pbuf"], 0, OFFS[8])
            mask_strips(hs["pbuf"], 0, 8)
            # TE work for the exp-A window: previous head's second-half PV
            if carry is not None:
                pv_half(carry, 1)
                carry = None
            # ---- ACT second half: tanh g4..g7, exp(strips 8-15)
            for g in range(4, 8):
                tanh_group(g, qk_group(h, g), hs["tbuf"])
            # hoist next head's first two QK groups (TE work + early tanh input)
            if h + 1 < HQ_PER_CORE:
                pending[(h + 1, 0)] = qk_group(h + 1, 0)
                pending[(h + 1, 1)] = qk_group(h + 1, 1)
            if h < HQ_PER_CORE - 1:
                exp_chunk(hs["tbuf"], hs["pbuf"], OFFS[8], TOT)
                mask_strips(hs["pbuf"], 8, NB)
                # TE work for the exp-B window: this head's first-half PV
                pv_half(hs, 0)
                carry = hs
            else:
                # last head: first half, then a fine-grained tail per strip-pair
                pv_half(hs, 0)
                pbuf, obuf, out_v = hs["pbuf"], hs["obuf"], hs["out_v"]
                ots = {2: None, 3: None}
                for g in range(4, 8):
                    kb0, kb1 = 2 * g, 2 * g + 1
                    exp_chunk(hs["tbuf"], pbuf, OFFS[kb0],
                              OFFS[kb1] + WIDTHS[kb1])
                    mask_strips(pbuf, kb0, kb1 + 1)
                    for qb in (kb0, kb1):
                        quad = qb // 4
                        if ots[quad] is None:
                            ots[quad] = opsum.tile(
                                [128, 4, 256], F32, name="ot", tag="ot"
                            )
                        pv_qb(pbuf, ots[quad], qb)
                    if g == 5:
                        normalize_quad(ots[2], obuf, 2)
                        nc.sync.dma_start(
                            out=out_v[:, 8:12, :], in_=obuf[:, 8:12, :]
                        )
                normalize_quad(ots[3], obuf, 3)
                nc.sync.dma_start(out=out_v[:, 12:14, :], in_=obuf[:, 12:14, :])
                nc.sync.dma_start(out=out_v[:, 14:NB, :], in_=obuf[:, 14:NB, :])
    return nc


_CACHED = None


def _build():
    global _CACHED
    if _CACHED is None:
        nc = bacc.Bacc()
        q = nc.dram_tensor("q", [HQ_PER_CORE, SQ, D], BF16, kind="ExternalInput")
        k = nc.dram_tensor("k", [SQ, D], BF16, kind="ExternalInput")
        v = nc.dram_tensor("v", [SQ, D], BF16, kind="ExternalInput")
        out = nc.dram_tensor("out", [HQ_PER_CORE, SQ, D], F32, kind="ExternalOutput")
        build_attention(nc, q[:], k[:], v[:], out[:])
        nc.finalize()
        _CACHED = nc
    return _CACHED


def make_in_maps(Q, K, V):
    import ml_dtypes

    Qn = np.asarray(Q).astype(ml_dtypes.bfloat16).reshape(32, SQ, D)
    Kn = np.asarray(K).astype(ml_dtypes.bfloat16).reshape(8, SQ, D)
    Vn = np.asarray(V).astype(ml_dtypes.bfloat16).reshape(8, SQ, D)
    return [
        {
            "q": np.ascontiguousarray(Qn[4 * c : 4 * c + 4]),
            "k": np.ascontiguousarray(Kn[c]),
            "v": np.ascontiguousarray(Vn[c]),
        }
        for c in range(N_CORES)
    ]


def kernel(Q, K, V):
    nc = _build()
    in_maps = make_in_maps(Q, K, V)
    res = run_bass_kernel_spmd(nc, in_maps, list(range(N_CORES))).results
    out = np.stack([res[c]["out"] for c in range(N_CORES)])  # [8,4,2048,128]
    return out.reshape(1, 32, SQ, D).astype(np.float32)


# revision 9
# speedup vs baseline: 1.2966x; 1.1516x over previous
"""Sliding-window GQA attention (softcap) on 8 trn2 NeuronCores.

Problem shapes (hardcoded):
  Q [1, 32, 2048, 128] bf16, K/V [1, 8, 2048, 128] bf16 -> out [1, 32, 2048, 128] f32
  causal, window_left=256, softcap=30, scale=1/sqrt(128), GQA group=4.

Sharding: core c owns kv-head c and query heads [4c, 4c+4). Each (b, h_kv)
slice is fully independent -> no collectives.

Per-core kernel, v2 (ACT-pipelined):
  ACT (tanh+exp over all 5760 score cols/head, ~11.6us/head busy) is the
  bottleneck engine; the schedule keeps it streaming:
  - a warmup activation at t=0 pulls the ~2.7us ACT table load into the DMA
    lead-in; K^T/Q^T transposes go on two HWDGE rings in parallel.
  - ACT order per head: [tanh g0..g3, exp(strips 0-7), tanh g4..g7,
    exp(strips 8-15)]; the two big exp chunks amortize the ~330-cycle
    per-instruction overhead.
  - software-pipelined carry: head h's second-half mask/PV/normalize/DMA is
    emitted inside head h+1's loop, so on the TensorE queue it lands in the
    exp-A(h+1) window and never delays the QK matmuls that feed tanh.
  - scores stay in the transposed S^T[k, q] layout (strip per key-block) so
    post-softmax P^T is directly the PV lhsT; softcap bounds scores at +-30
    so exp uses the constant shift 30 (no row max).
  - PV accumulates O (+ row-sum via a ones-column in V) into [128,4,256]
    psum quads; normalize is one recip + one broadcast-mul per quad.
  - the last head's tail runs exp/PV/normalize/DMA per strip-pair.
"""

import math
from contextlib import ExitStack

import numpy as np

import concourse.bacc as bacc
import concourse.bass as bass
import concourse.mybir as mybir
import concourse.tile as tile
from concourse.bass import MemorySpace
from concourse.bass_utils import run_bass_kernel_spmd

BF16 = mybir.dt.bfloat16
F32 = mybir.dt.float32

N_CORES = 8
HQ_PER_CORE = 4  # GQA group size
SQ = 2048
D = 128
NB = SQ // 128  # 16 key/query blocks
SCALE = 1.0 / math.sqrt(128.0)
SOFTCAP = 30.0

# strip widths: key-block kb sees q-columns [kb*128, kb*128 + W[kb])
WIDTHS = [min(384, SQ - kb * 128) for kb in range(NB)]
OFFS = [sum(WIDTHS[:kb]) for kb in range(NB)]
TOT = sum(WIDTHS)  # 5760 score columns per head


def build_attention(nc: bass.Bass, q, k, v, out):
    """q [4,2048,128] bf16; k,v [2048,128] bf16; out [4,2048,128] f32 (DRAM APs)."""
    with ExitStack() as ctx:
        tc = ctx.enter_context(tile.TileContext(nc))
        consts = ctx.enter_context(tc.tile_pool(name="consts", bufs=1))
        qt_pool = ctx.enter_context(tc.tile_pool(name="qt", bufs=3))
        t_pool = ctx.enter_context(tc.tile_pool(name="tbuf", bufs=2))
        p_pool = ctx.enter_context(tc.tile_pool(name="pbuf", bufs=2))
        o_pool = ctx.enter_context(tc.tile_pool(name="obuf", bufs=2))
        r_pool = ctx.enter_context(tc.tile_pool(name="rtile", bufs=4))
        spsum = ctx.enter_context(
            tc.tile_pool(name="spsum", bufs=3, space=MemorySpace.PSUM)
        )
        opsum = ctx.enter_context(
            tc.tile_pool(name="opsum", bufs=2, space=MemorySpace.PSUM)
        )

        # ---- t=0: ACT table-load warmup (exp set includes tanh). The memset
        # runs on DVE (gpsimd's first instruction pays a ~6us IRAM load).
        warm = consts.tile([128, 2], F32)
        nc.vector.memset(warm[:, 0:1], 0.0)
        nc.scalar.activation(
            out=warm[:, 1:2], in_=warm[:, 0:1],
            func=mybir.ActivationFunctionType.Exp,
        )
        negcap = consts.tile([128, 1], F32)
        nc.vector.memset(negcap, -SOFTCAP)

        # ---- input staging: K^T is the single scalar-ring trigger (issued at
        # t=0 while the ACT queue is otherwise empty -- mid-stream triggers on
        # the scalar queue cost ~2us and stall the activation pipeline).
        # Q^T transposes + V + output stores all go on the sync ring.
        kt = consts.tile([128, SQ], BF16)
        qts = [
            qt_pool.tile([128, SQ], BF16, name=f"qt{h}", tag="qt")
            for h in range(HQ_PER_CORE)
        ]
        nc.scalar.dma_start_transpose(out=kt, in_=k)
        nc.sync.dma_start_transpose(out=qts[0], in_=q[0])
        nc.sync.dma_start_transpose(out=qts[1], in_=q[1])

        def kt_blk(kb):
            return kt[:, kb * 128 : (kb + 1) * 128]

        def qt_rhs(h, kb, w):
            return qts[h][:, kb * 128 : kb * 128 + w]

        # V blocks + ones column (sync ring, after the transposes)
        vt = consts.tile([128, NB, 129], BF16)
        nc.vector.memset(vt[:, :, 128:129], 1.0)
        nc.sync.dma_start(
            out=vt[:, :, 0:128], in_=v.rearrange("(t p) d -> p t d", p=128)
        )
        # band masks, combined [128, 2, 128]: slot 0 keeps c >= kr (upper tri
        # incl diag, strip block 0), slot 1 keeps c <= kr (lower tri, block 2)
        muL = consts.tile([128, 2, 128], BF16)
        nc.gpsimd.memset(muL, 1.0)
        nc.gpsimd.affine_select(
            out=muL[:, 0, :], in_=muL[:, 0, :], compare_op=mybir.AluOpType.is_ge,
            fill=0.0, base=0, pattern=[[1, 128]], channel_multiplier=-1,
        )
        nc.gpsimd.affine_select(
            out=muL[:, 1, :], in_=muL[:, 1, :], compare_op=mybir.AluOpType.is_ge,
            fill=0.0, base=0, pattern=[[-1, 128]], channel_multiplier=1,
        )

        def qk_group(h, g):
            """Scores for strips (2g, 2g+1) -> one 2-bank psum tile."""
            kb0, kb1 = 2 * g, 2 * g + 1
            sp = spsum.tile([128, 1024], F32, name="sp", tag="sp")
            for j, kb in enumerate((kb0, kb1)):
                w = WIDTHS[kb]
                nc.tensor.matmul(
                    out=sp[:, j * 512 : j * 512 + w],
                    lhsT=kt_blk(kb),
                    rhs=qt_rhs(h, kb, w),
                    start=True,
                    stop=True,
                )
            return sp

        def tanh_group(g, sp, tbuf):
            kb0, kb1 = 2 * g, 2 * g + 1
            if WIDTHS[kb0] == WIDTHS[kb1]:
                w = WIDTHS[kb0]
                src = sp[:].rearrange("p (g x) -> p g x", g=2)[:, :, 0:w]
                dst = tbuf[:, OFFS[kb0] : OFFS[kb0] + 2 * w].rearrange(
                    "p (g x) -> p g x", g=2
                )
                nc.scalar.activation(
                    out=dst, in_=src,
                    func=mybir.ActivationFunctionType.Tanh,
                    scale=SCALE / SOFTCAP,
                )
            else:
                for j, kb in enumerate((kb0, kb1)):
                    w = WIDTHS[kb]
                    nc.scalar.activation(
                        out=tbuf[:, OFFS[kb] : OFFS[kb] + w],
                        in_=sp[:, j * 512 : j * 512 + w],
                        func=mybir.ActivationFunctionType.Tanh,
                        scale=SCALE / SOFTCAP,
                    )

        def exp_chunk(tbuf, pbuf, lo, hi):
            nc.scalar.activation(
                out=pbuf[:, lo:hi], in_=tbuf[:, lo:hi],
                func=mybir.ActivationFunctionType.Exp,
                scale=SOFTCAP, bias=negcap,
            )

        def mask_strips(pbuf, kb_lo, kb_hi):
            """Zero invalid triangles of strips [kb_lo, kb_hi)."""
            for kb in range(kb_lo, kb_hi):
                off = OFFS[kb]
                if WIDTHS[kb] == 384:
                    view = pbuf[:, off : off + 384].rearrange(
                        "p (a x) -> p a x", x=128
                    )[:, ::2, :]
                    nc.vector.tensor_mul(out=view, in0=view, in1=muL)
                else:
                    nc.vector.tensor_mul(
                        out=pbuf[:, off : off + 128],
                        in0=pbuf[:, off : off + 128],
                        in1=muL[:, 0, :],
                    )

        def pv_qb(pbuf, otile, qb):
            """Accumulate O[qb] (+ rowsum col 128) into otile slot qb%2."""
            kbs = [kb for kb in (qb - 2, qb - 1, qb) if kb >= 0]
            for kb in kbs:
                j = qb - kb
                nc.tensor.matmul(
                    out=otile[:, qb % 2, 0:129],
                    lhsT=pbuf[:, OFFS[kb] + j * 128 : OFFS[kb] + (j + 1) * 128],
                    rhs=vt[:, kb, :],
                    start=(kb == kbs[0]),
                    stop=(kb == qb),
                )

        def normalize_pair(otile, hs, pair):
            """Normalize qb pair (2*pair, 2*pair+1), write obuf, DMA out."""
            obuf, out_v = hs["obuf"], hs["out_v"]
            rt = r_pool.tile([128, 2], F32)
            nc.vector.reciprocal(out=rt, in_=otile[:, :, 128])
            nc.vector.tensor_mul(
                out=obuf[:, 2 * pair : 2 * pair + 2, :],
                in0=otile[:, :, 0:128],
                in1=rt.to_broadcast([128, 2, 128]),
            )
            nc.sync.dma_start(
                out=out_v[:, 2 * pair : 2 * pair + 2, :],
                in_=obuf[:, 2 * pair : 2 * pair + 2, :],
            )

        def pv_half(hs, half):
            """PV/normalize/DMA for strips 8*half..8*half+8 (mask already done)."""
            pbuf = hs["pbuf"]
            for pair in range(4 * half, 4 * half + 4):
                ot = opsum.tile([128, 2, 132], F32, name="ot", tag="ot")
                pv_qb(pbuf, ot, 2 * pair)
                pv_qb(pbuf, ot, 2 * pair + 1)
                normalize_pair(ot, hs, pair)

        pending = {}
        carry = None  # head state whose second half still needs PV/out
        for h in range(HQ_PER_CORE):
            if h + 2 < HQ_PER_CORE:
                nc.sync.dma_start_transpose(out=qts[h + 2], in_=q[h + 2])
            hs = {
                "h": h,
                "tbuf": t_pool.tile([128, TOT], F32, name="tbuf", tag="tbuf"),
                "pbuf": p_pool.tile([128, TOT], BF16, name="pbuf", tag="pbuf"),
                "obuf": o_pool.tile([128, NB, 128], F32, name="obuf", tag="obuf"),
                "out_v": out[h].rearrange("(qb p) d -> p qb d", p=128),
            }
            # ---- ACT first half: tanh g0..g3, exp(strips 0-7), mask right away
            for g in range(4):
                sp = pending.pop((h, g), None)
                if sp is None:
                    sp = qk_group(h, g)
                tanh_group(g, sp, hs["tbuf"])
            exp_chunk(hs["tbuf"], hs["pbuf"], 0, OFFS[8])
            mask_strips(hs["pbuf"], 0, 8)
            # TE work for the exp-A window: previous head's second-half PV
            if carry is not None:
                pv_half(carry, 1)
                carry = None
            # ---- ACT second half: tanh g4..g7, exp(strips 8-15)
            for g in range(4, 8):
                tanh_group(g, qk_group(h, g), hs["tbuf"])
            # hoist next head's first two QK groups (TE work + early tanh input)
            if h + 1 < HQ_PER_CORE:
                pending[(h + 1, 0)] = qk_group(h + 1, 0)
                pending[(h + 1, 1)] = qk_group(h + 1, 1)
            if h < HQ_PER_CORE - 1:
                exp_chunk(hs["tbuf"], hs["pbuf"], OFFS[8], TOT)
                mask_strips(hs["pbuf"], 8, NB)
                # TE work for the exp-B window: this head's first-half PV
                pv_half(hs, 0)
                carry = hs
            else:
                # last head: first half, then a fine-grained tail per strip-pair
                pv_half(hs, 0)
                pbuf, obuf, out_v = hs["pbuf"], hs["obuf"], hs["out_v"]
                for g in range(4, 8):
                    kb0, kb1 = 2 * g, 2 * g + 1
                    exp_chunk(hs["tbuf"], pbuf, OFFS[kb0],
                              OFFS[kb1] + WIDTHS[kb1])
                    mask_strips(pbuf, kb0, kb1 + 1)
                    ot = opsum.tile([128, 2, 132], F32, name="ot", tag="ot")
                    pv_qb(pbuf, ot, kb0)
                    pv_qb(pbuf, ot, kb1)
                    # final pieces ride the idle scalar ring once ACT is done
                    rt = r_pool.tile([128, 2], F32)
                    nc.vector.reciprocal(out=rt, in_=ot[:, :, 128])
                    nc.vector.tensor_mul(
                        out=obuf[:, kb0 : kb0 + 2, :],
                        in0=ot[:, :, 0:128],
                        in1=rt.to_broadcast([128, 2, 128]),
                    )
                    eng = nc.sync if g < 6 else nc.scalar
                    eng.dma_start(
                        out=out_v[:, kb0 : kb0 + 2, :],
                        in_=obuf[:, kb0 : kb0 + 2, :],
                    )
    return nc


_CACHED = None


def _build():
    global _CACHED
    if _CACHED is None:
        nc = bacc.Bacc()
        q = nc.dram_tensor("q", [HQ_PER_CORE, SQ, D], BF16, kind="ExternalInput")
        k = nc.dram_tensor("k", [SQ, D], BF16, kind="ExternalInput")
        v = nc.dram_tensor("v", [SQ, D], BF16, kind="ExternalInput")
        out = nc.dram_tensor("out", [HQ_PER_CORE, SQ, D], F32, kind="ExternalOutput")
        build_attention(nc, q[:], k[:], v[:], out[:])
        nc.finalize()
        _CACHED = nc
    return _CACHED


def make_in_maps(Q, K, V):
    import ml_dtypes

    Qn = np.asarray(Q).astype(ml_dtypes.bfloat16).reshape(32, SQ, D)
    Kn = np.asarray(K).astype(ml_dtypes.bfloat16).reshape(8, SQ, D)
    Vn = np.asarray(V).astype(ml_dtypes.bfloat16).reshape(8, SQ, D)
    return [
        {
            "q": np.ascontiguousarray(Qn[4 * c : 4 * c + 4]),
            "k": np.ascontiguousarray(Kn[c]),
            "v": np.ascontiguousarray(Vn[c]),
        }
        for c in range(N_CORES)
    ]


def kernel(Q, K, V):
    nc = _build()
    in_maps = make_in_maps(Q, K, V)
    res = run_bass_kernel_spmd(nc, in_maps, list(range(N_CORES))).results
    out = np.stack([res[c]["out"] for c in range(N_CORES)])  # [8,4,2048,128]
    return out.reshape(1, 32, SQ, D).astype(np.float32)


# revision 10
# speedup vs baseline: 1.3062x; 1.0074x over previous
"""Sliding-window GQA attention (softcap) on 8 trn2 NeuronCores.

Problem shapes (hardcoded):
  Q [1, 32, 2048, 128] bf16, K/V [1, 8, 2048, 128] bf16 -> out [1, 32, 2048, 128] f32
  causal, window_left=256, softcap=30, scale=1/sqrt(128), GQA group=4.

Sharding: core c owns kv-head c and query heads [4c, 4c+4). Each (b, h_kv)
slice is fully independent -> no collectives.

Per-core kernel, v2 (ACT-pipelined):
  ACT (tanh+exp over all 5760 score cols/head, ~11.6us/head busy) is the
  bottleneck engine; the schedule keeps it streaming:
  - a warmup activation at t=0 pulls the ~2.7us ACT table load into the DMA
    lead-in; K^T/Q^T transposes go on two HWDGE rings in parallel.
  - ACT order per head: [tanh g0..g3, exp(strips 0-7), tanh g4..g7,
    exp(strips 8-15)]; the two big exp chunks amortize the ~330-cycle
    per-instruction overhead.
  - software-pipelined carry: head h's second-half mask/PV/normalize/DMA is
    emitted inside head h+1's loop, so on the TensorE queue it lands in the
    exp-A(h+1) window and never delays the QK matmuls that feed tanh.
  - scores stay in the transposed S^T[k, q] layout (strip per key-block) so
    post-softmax P^T is directly the PV lhsT; softcap bounds scores at +-30
    so exp uses the constant shift 30 (no row max).
  - PV accumulates O (+ row-sum via a ones-column in V) into [128,4,256]
    psum quads; normalize is one recip + one broadcast-mul per quad.
  - the last head's tail runs exp/PV/normalize/DMA per strip-pair.
"""

import math
from contextlib import ExitStack

import numpy as np

import concourse.bacc as bacc
import concourse.bass as bass
import concourse.mybir as mybir
import concourse.tile as tile
from concourse.bass import MemorySpace
from concourse.bass_utils import run_bass_kernel_spmd

BF16 = mybir.dt.bfloat16
F32 = mybir.dt.float32

N_CORES = 8
HQ_PER_CORE = 4  # GQA group size
SQ = 2048
D = 128
NB = SQ // 128  # 16 key/query blocks
SCALE = 1.0 / math.sqrt(128.0)
SOFTCAP = 30.0

# strip widths: key-block kb sees q-columns [kb*128, kb*128 + W[kb])
WIDTHS = [min(384, SQ - kb * 128) for kb in range(NB)]
OFFS = [sum(WIDTHS[:kb]) for kb in range(NB)]
TOT = sum(WIDTHS)  # 5760 score columns per head


def build_attention(nc: bass.Bass, q, k, v, mask, out):
    """q [4,128,2048] bf16 (pre-transposed); k [128,2048] bf16 (pre-transposed);
    v [2048,129] bf16 (ones col appended); mask [128,2,128] bf16;
    out [4,2048,128] f32 (DRAM APs)."""
    with ExitStack() as ctx:
        tc = ctx.enter_context(tile.TileContext(nc))
        consts = ctx.enter_context(tc.tile_pool(name="consts", bufs=1))
        qt_pool = ctx.enter_context(tc.tile_pool(name="qt", bufs=3))
        t_pool = ctx.enter_context(tc.tile_pool(name="tbuf", bufs=2))
        p_pool = ctx.enter_context(tc.tile_pool(name="pbuf", bufs=2))
        o_pool = ctx.enter_context(tc.tile_pool(name="obuf", bufs=2))
        r_pool = ctx.enter_context(tc.tile_pool(name="rtile", bufs=4))
        spsum = ctx.enter_context(
            tc.tile_pool(name="spsum", bufs=3, space=MemorySpace.PSUM)
        )
        opsum = ctx.enter_context(
            tc.tile_pool(name="opsum", bufs=2, space=MemorySpace.PSUM)
        )

        # ---- t=0: ACT table-load warmup (exp set includes tanh). The memset
        # runs on DVE (gpsimd's first instruction pays a ~6us IRAM load).
        warm = consts.tile([128, 2], F32)
        nc.vector.memset(warm[:, 0:1], 0.0)
        nc.scalar.activation(
            out=warm[:, 1:2], in_=warm[:, 0:1],
            func=mybir.ActivationFunctionType.Exp,
        )
        negcap = consts.tile([128, 1], F32)
        nc.vector.memset(negcap, -SOFTCAP)

        # ---- input staging. Q^T/K^T come pre-transposed from the host and
        # V arrives with the ones-column appended, so every load is a plain
        # wide DMA (no xbar transposes, no gpsimd, no memsets). K^T rides the
        # scalar ring at t=0 (its only trigger); everything else is sync-ring.
        kt = consts.tile([128, SQ], BF16)
        qts = [
            qt_pool.tile([128, SQ], BF16, name=f"qt{h}", tag="qt")
            for h in range(HQ_PER_CORE)
        ]
        nc.scalar.dma_start(out=kt, in_=k)
        nc.sync.dma_start(out=qts[0], in_=q[0])
        nc.sync.dma_start(out=qts[1], in_=q[1])

        def kt_blk(kb):
            return kt[:, kb * 128 : (kb + 1) * 128]

        def qt_rhs(h, kb, w):
            return qts[h][:, kb * 128 : kb * 128 + w]

        # V blocks + host-appended ones column
        vt = consts.tile([128, NB, 129], BF16)
        nc.sync.dma_start(
            out=vt, in_=v.rearrange("(t p) d -> p t d", p=128)
        )
        # band masks from the host, [128, 2, 128]: slot 0 keeps c >= kr (upper
        # tri incl diag, strip block 0), slot 1 keeps c <= kr (strip block 2)
        muL = consts.tile([128, 2, 128], BF16)
        nc.sync.dma_start(out=muL, in_=mask)

        def qk_group(h, g):
            """Scores for strips (2g, 2g+1) -> one 2-bank psum tile."""
            kb0, kb1 = 2 * g, 2 * g + 1
            sp = spsum.tile([128, 1024], F32, name="sp", tag="sp")
            for j, kb in enumerate((kb0, kb1)):
                w = WIDTHS[kb]
                nc.tensor.matmul(
                    out=sp[:, j * 512 : j * 512 + w],
                    lhsT=kt_blk(kb),
                    rhs=qt_rhs(h, kb, w),
                    start=True,
                    stop=True,
                )
            return sp

        def tanh_group(g, sp, tbuf):
            kb0, kb1 = 2 * g, 2 * g + 1
            if WIDTHS[kb0] == WIDTHS[kb1]:
                w = WIDTHS[kb0]
                src = sp[:].rearrange("p (g x) -> p g x", g=2)[:, :, 0:w]
                dst = tbuf[:, OFFS[kb0] : OFFS[kb0] + 2 * w].rearrange(
                    "p (g x) -> p g x", g=2
                )
                nc.scalar.activation(
                    out=dst, in_=src,
                    func=mybir.ActivationFunctionType.Tanh,
                    scale=SCALE / SOFTCAP,
                )
            else:
                for j, kb in enumerate((kb0, kb1)):
                    w = WIDTHS[kb]
                    nc.scalar.activation(
                        out=tbuf[:, OFFS[kb] : OFFS[kb] + w],
                        in_=sp[:, j * 512 : j * 512 + w],
                        func=mybir.ActivationFunctionType.Tanh,
                        scale=SCALE / SOFTCAP,
                    )

        def exp_chunk(tbuf, pbuf, lo, hi):
            nc.scalar.activation(
                out=pbuf[:, lo:hi], in_=tbuf[:, lo:hi],
                func=mybir.ActivationFunctionType.Exp,
                scale=SOFTCAP, bias=negcap,
            )

        def mask_strips(pbuf, kb_lo, kb_hi):
            """Zero invalid triangles of strips [kb_lo, kb_hi)."""
            for kb in range(kb_lo, kb_hi):
                off = OFFS[kb]
                if WIDTHS[kb] == 384:
                    view = pbuf[:, off : off + 384].rearrange(
                        "p (a x) -> p a x", x=128
                    )[:, ::2, :]
                    nc.vector.tensor_mul(out=view, in0=view, in1=muL)
                else:
                    nc.vector.tensor_mul(
                        out=pbuf[:, off : off + 128],
                        in0=pbuf[:, off : off + 128],
                        in1=muL[:, 0, :],
                    )

        def pv_qb(pbuf, otile, qb):
            """Accumulate O[qb] (+ rowsum col 128) into otile slot qb%2."""
            kbs = [kb for kb in (qb - 2, qb - 1, qb) if kb >= 0]
            for kb in kbs:
                j = qb - kb
                nc.tensor.matmul(
                    out=otile[:, qb % 2, 0:129],
                    lhsT=pbuf[:, OFFS[kb] + j * 128 : OFFS[kb] + (j + 1) * 128],
                    rhs=vt[:, kb, :],
                    start=(kb == kbs[0]),
                    stop=(kb == qb),
                )

        def normalize_pair(otile, hs, pair):
            """Normalize qb pair (2*pair, 2*pair+1), write obuf, DMA out."""
            obuf, out_v = hs["obuf"], hs["out_v"]
            rt = r_pool.tile([128, 2], F32)
            nc.vector.reciprocal(out=rt, in_=otile[:, :, 128])
            nc.vector.tensor_mul(
                out=obuf[:, 2 * pair : 2 * pair + 2, :],
                in0=otile[:, :, 0:128],
                in1=rt.to_broadcast([128, 2, 128]),
            )
            nc.sync.dma_start(
                out=out_v[:, 2 * pair : 2 * pair + 2, :],
                in_=obuf[:, 2 * pair : 2 * pair + 2, :],
            )

        def pv_half(hs, half):
            """PV/normalize/DMA for strips 8*half..8*half+8 (mask already done)."""
            pbuf = hs["pbuf"]
            for pair in range(4 * half, 4 * half + 4):
                ot = opsum.tile([128, 2, 132], F32, name="ot", tag="ot")
                pv_qb(pbuf, ot, 2 * pair)
                pv_qb(pbuf, ot, 2 * pair + 1)
                normalize_pair(ot, hs, pair)

        pending = {}
        carry = None  # head state whose second half still needs PV/out
        for h in range(HQ_PER_CORE):
            if h + 2 < HQ_PER_CORE:
                nc.sync.dma_start(out=qts[h + 2], in_=q[h + 2])
            hs = {
                "h": h,
                "tbuf": t_pool.tile([128, TOT], F32, name="tbuf", tag="tbuf"),
                "pbuf": p_pool.tile([128, TOT], BF16, name="pbuf", tag="pbuf"),
                "obuf": o_pool.tile([128, NB, 128], F32, name="obuf", tag="obuf"),
                "out_v": out[h].rearrange("(qb p) d -> p qb d", p=128),
            }
            # ---- ACT first half: tanh g0..g3, exp(strips 0-7), mask right away
            for g in range(4):
                sp = pending.pop((h, g), None)
                if sp is None:
                    sp = qk_group(h, g)
                tanh_group(g, sp, hs["tbuf"])
            exp_chunk(hs["tbuf"], hs["pbuf"], 0, OFFS[8])
            mask_strips(hs["pbuf"], 0, 8)
            # TE work for the exp-A window: previous head's second-half PV
            if carry is not None:
                pv_half(carry, 1)
                carry = None
            # ---- ACT second half: tanh g4..g7, exp(strips 8-15)
            for g in range(4, 8):
                tanh_group(g, qk_group(h, g), hs["tbuf"])
            # hoist next head's first two QK groups (TE work + early tanh input)
            if h + 1 < HQ_PER_CORE:
                pending[(h + 1, 0)] = qk_group(h + 1, 0)
                pending[(h + 1, 1)] = qk_group(h + 1, 1)
            if h < HQ_PER_CORE - 1:
                exp_chunk(hs["tbuf"], hs["pbuf"], OFFS[8], TOT)
                mask_strips(hs["pbuf"], 8, NB)
                # TE work for the exp-B window: this head's first-half PV
                pv_half(hs, 0)
                carry = hs
            else:
                # last head: first half, then a fine-grained tail per strip-pair
                pv_half(hs, 0)
                pbuf, obuf, out_v = hs["pbuf"], hs["obuf"], hs["out_v"]
                deferred = []
                for g in range(4, 8):
                    kb0, kb1 = 2 * g, 2 * g + 1
                    exp_chunk(hs["tbuf"], pbuf, OFFS[kb0],
                              OFFS[kb1] + WIDTHS[kb1])
                    mask_strips(pbuf, kb0, kb1 + 1)
                    ot = opsum.tile([128, 2, 132], F32, name="ot", tag="ot")
                    pv_qb(pbuf, ot, kb0)
                    pv_qb(pbuf, ot, kb1)
                    rt = r_pool.tile([128, 2], F32)
                    nc.vector.reciprocal(out=rt, in_=ot[:, :, 128])
                    nc.vector.tensor_mul(
                        out=obuf[:, kb0 : kb0 + 2, :],
                        in0=ot[:, :, 0:128],
                        in1=rt.to_broadcast([128, 2, 128]),
                    )
                    if g < 6:
                        nc.sync.dma_start(
                            out=out_v[:, kb0 : kb0 + 2, :],
                            in_=obuf[:, kb0 : kb0 + 2, :],
                        )
                    else:
                        # final pieces ride the now-idle scalar ring; their
                        # triggers are emitted only after all ACT work so they
                        # never block an activation in the FIFO
                        deferred.append(kb0)
                for kb0 in deferred:
                    nc.scalar.dma_start(
                        out=out_v[:, kb0 : kb0 + 2, :],
                        in_=obuf[:, kb0 : kb0 + 2, :],
                    )
    return nc


_CACHED = None


def _build():
    global _CACHED
    if _CACHED is None:
        nc = bacc.Bacc()
        q = nc.dram_tensor("q", [HQ_PER_CORE, D, SQ], BF16, kind="ExternalInput")
        k = nc.dram_tensor("k", [D, SQ], BF16, kind="ExternalInput")
        v = nc.dram_tensor("v", [SQ, D + 1], BF16, kind="ExternalInput")
        mask = nc.dram_tensor("mask", [128, 2, 128], BF16, kind="ExternalInput")
        out = nc.dram_tensor("out", [HQ_PER_CORE, SQ, D], F32, kind="ExternalOutput")
        build_attention(nc, q[:], k[:], v[:], mask[:], out[:])
        nc.finalize()
        _CACHED = nc
    return _CACHED


def make_in_maps(Q, K, V):
    import ml_dtypes

    Qt = np.asarray(Q).astype(ml_dtypes.bfloat16).reshape(32, SQ, D)
    Qt = np.ascontiguousarray(Qt.transpose(0, 2, 1))  # [32, 128, 2048]
    Kt = np.asarray(K).astype(ml_dtypes.bfloat16).reshape(8, SQ, D)
    Kt = np.ascontiguousarray(Kt.transpose(0, 2, 1))  # [8, 128, 2048]
    Vn = np.asarray(V).astype(ml_dtypes.bfloat16).reshape(8, SQ, D)
    Va = np.concatenate(
        [Vn, np.ones((8, SQ, 1), dtype=ml_dtypes.bfloat16)], axis=2
    )  # [8, 2048, 129]
    r = np.arange(128)
    muL = np.zeros((128, 2, 128), dtype=ml_dtypes.bfloat16)
    muL[:, 0, :] = (r[None, :] >= r[:, None])  # strip block 0: keep c >= kr
    muL[:, 1, :] = (r[None, :] <= r[:, None])  # strip block 2: keep c <= kr
    return [
        {
            "q": np.ascontiguousarray(Qt[4 * c : 4 * c + 4]),
            "k": np.ascontiguousarray(Kt[c]),
            "v": np.ascontiguousarray(Va[c]),
            "mask": muL,
        }
        for c in range(N_CORES)
    ]


def kernel(Q, K, V):
    nc = _build()
    in_maps = make_in_maps(Q, K, V)
    res = run_bass_kernel_spmd(nc, in_maps, list(range(N_CORES))).results
    out = np.stack([res[c]["out"] for c in range(N_CORES)])  # [8,4,2048,128]
    return out.reshape(1, 32, SQ, D).astype(np.float32)
